# revision 2
# baseline (speedup 1.0000x reference)
"""Trainium2 Bass kernel for nn_CrossAttention (B=128, C=1024) — v2.

Math (per sample b):
    crossed_i = f(t_i),  f(x) = sum_j v_j e^{x v_j} / sum_j e^{x v_j}
f is a smooth scalar function per sample -> barycentric interpolation from
K=64 Chebyshev(2nd-kind-ish) nodes on [-a, a] instead of the 1024x1024
softmax (16x less exp work; all reductions become matmuls).

Device pipeline per core (16 samples):
  1. P_s[j,(c,k)] = v_j * x_k outer product: one K=64 matmul per sample
     (lhsT = [v_hi; v_lo] f32r split rows -> exponent exact to ~1e-5;
     rhs = constant block-diagonal x pattern).  exp on ScalarE from PSUM.
  2. N_k, D_k: quad-packed accumulating matmuls (lhsT = [vT|1] cols,
     rhs = E 4-sample segments, moving 256).  f = N/D via DVE recip_approx.
  3. A[(z,k), i] = t~ - x via K=33 matmul (t_hi, t_lo, ones rows; exact),
     R = 1/A via custom-DVE recip_approx_fast written directly as f32r.
  4. num/den = matmul([w*f | w], R); crossed = num * recip(den); h = v+crossed.
  5. y = h @ Wp + bp (Wp host-rearranged so the 4MB DMA is 128 contiguous
     32KB descriptors; bias via extra K=2 matmul rows), LeakyReLU on DVE.
"""

import os
import numpy as np

import concourse.bass as bass
import concourse.bacc as bacc
import concourse.tile as tile
from concourse import mybir
from concourse.bass_utils import run_bass_kernel_spmd
from concourse.masks import make_identity
from concourse.dve_ops import RECIP_APPROX_FAST_CONSTS, RECIPROCAL_APPROX_FAST

B, C = 128, 1024
N_CORES = 8
SPC = B // N_CORES          # samples per core (16)
NCH = C // 128              # j-chunks (8)
K = 64                      # interpolation nodes
DOM = 5.3                   # node domain [-DOM, DOM]
BN_EPS = 1e-5
SLOPE = 0.1
F32 = mybir.dt.float32
F32R = mybir.dt.float32r
EXP = mybir.ActivationFunctionType.Exp


def _trunc10(x):
    """Truncate fp32 mantissa to 10 explicit bits (f32r-exact values)."""
    xi = np.asarray(x, np.float32).view(np.uint32)
    return (xi & np.uint32(0xFFFFE000)).view(np.float32)


def _f32r_split(x):
    """x ~= hi + lo, both 10-bit-mantissa (f32r-exact); ~20-bit total."""
    x = np.asarray(x, np.float32)
    hi = _trunc10(x)
    lo = _trunc10(x - hi)
    return hi, lo


def _nodes():
    k = np.arange(K)
    x = _trunc10(DOM * np.cos(np.pi * k / (K - 1))).astype(np.float64)
    assert len(np.unique(x)) == K
    w = np.ones(K)
    for i in range(K):
        w[i] = 1.0 / np.prod(np.delete(x[i] - x, i))
    w = w / np.abs(w).max()
    w = _trunc10(w).astype(np.float64)
    return x.astype(np.float32), w.astype(np.float32)


X_NODES, W_BARY = _nodes()


def _host_consts():
    """Data-independent constant blobs (DMA'd once per core)."""
    x, w = X_NODES, W_BARY
    # CXG [64, 4, 512]: outer-product rhs; rows r<32 pair with v_hi,
    # rows 32-63 with v_lo (same pattern); r%32 == 8i+c selects slot i chunk c
    cxg = np.zeros((64, 4, 512), np.float32)
    for r in range(64):
        i, c = (r % 32) // 8, (r % 32) % 8
        cxg[r, i, c * 64:(c + 1) * 64] = x
    cxg = cxg.reshape(64, 2048)

    # CST [128, 1216]:
    #  cols 128p..128p+128 (p<8): CA33 lhsT for pair p (rows 0-32):
    #    TINT rows: s = t_hi[s], 16+s = t_lo[s], 32 = ones
    #  cols 1024-1055: LWC (4j+1 = wcolA, 4j+3 = wcolB)
    #  cols 1056, 1057: wcolA, wcolB
    #  cols 1058-1073: ones [2, 16] (bias matmul lhsT, rows 0-1)
    #  cols 1088-1215: ones [128, 128] (vto init)
    cst = np.zeros((128, 1216), np.float32)
    for p in range(8):
        c0 = 128 * p
        cst[2 * p + 0, c0:c0 + 64] = 1.0       # tA hi
        cst[16 + 2 * p + 0, c0:c0 + 64] = 1.0  # tA lo
        cst[2 * p + 1, c0 + 64:c0 + 128] = 1.0       # tB hi
        cst[16 + 2 * p + 1, c0 + 64:c0 + 128] = 1.0  # tB lo
        cst[32, c0:c0 + 64] = -x
        cst[32, c0 + 64:c0 + 128] = -x
    wca = np.zeros(128, np.float32); wca[0:64] = w
    wcb = np.zeros(128, np.float32); wcb[64:128] = w
    for j in range(8):
        cst[:, 1024 + 4 * j + 1] = wca
        cst[:, 1024 + 4 * j + 3] = wcb
    cst[:, 1056] = wca
    cst[:, 1057] = wcb
    cst[0:2, 1058:1074] = 1.0
    cst[:, 1088:1216] = 1.0
    return cxg, cst


CXG_HOST, CST_HOST = _host_consts()


def build_nc():
    nc = bacc.Bacc("TRN2", target_bir_lowering=False, debug=False,
                   num_devices=N_CORES)
    v_d = nc.dram_tensor("v", [SPC, C], F32, kind="ExternalInput").ap()
    vhl_d = nc.dram_tensor("vhl", [4, 64, 128], F32R, kind="ExternalInput").ap()
    tint_d = nc.dram_tensor("tint", [33, C], F32R, kind="ExternalInput").ap()
    w_d = nc.dram_tensor("w", [128, NCH * C], F32R, kind="ExternalInput").ap()
    bp_d = nc.dram_tensor("bp", [3, C], F32R, kind="ExternalInput").ap()
    cxg_d = nc.dram_tensor("cxg", [64, 2048], F32R, kind="ExternalInput").ap()
    cst_d = nc.dram_tensor("cst", [128, 1216], F32R, kind="ExternalInput").ap()
    o_d = nc.dram_tensor("o", [SPC, C], F32, kind="ExternalOutput").ap()
    dbg = {}
    if os.environ.get("KERNEL_DEBUG"):
        dbg["e0"] = nc.dram_tensor("dbg_e0", [128, 512], F32, kind="ExternalOutput").ap()
        dbg["r0"] = nc.dram_tensor("dbg_r0", [128, C], F32, kind="ExternalOutput").ap()
        dbg["ndr"] = nc.dram_tensor("dbg_ndr", [SPC, 2 * K], F32, kind="ExternalOutput").ap()
        dbg["num"] = nc.dram_tensor("dbg_num", [SPC, C], F32, kind="ExternalOutput").ap()
        dbg["den"] = nc.dram_tensor("dbg_den", [SPC, C], F32, kind="ExternalOutput").ap()
        dbg["ndsb"] = nc.dram_tensor("dbg_ndsb", [8, 1024], F32, kind="ExternalOutput").ap()
        dbg["ndx"] = nc.dram_tensor("dbg_ndx", [8, 256], F32, kind="ExternalOutput").ap()

    with tile.TileContext(nc) as tc:
        _body(nc, tc, v_d, vhl_d, tint_d, w_d, bp_d, cxg_d, cst_d, o_d, dbg)
    nc.compile()
    return nc


def _recip_fast(nc, out, in_):
    c = RECIP_APPROX_FAST_CONSTS
    nc.vector._custom_dve(RECIPROCAL_APPROX_FAST, out=out, in0=in_,
                          s0=c["s0"], s1=c["s1"], imm2=c["imm2"])


def _body(nc, tc, v_d, vhl_d, tint_d, w_d, bp_d, cxg_d, cst_d, o_d, dbg=None):
    COPY = mybir.ActivationFunctionType.Copy
    with (
        tc.tile_pool(name="sing", bufs=1) as sing,
        tc.tile_pool(name="pps", bufs=2, space="PSUM") as pps,
        tc.tile_pool(name="ndps", bufs=1, space="PSUM") as ndps,
        tc.tile_pool(name="bigps", bufs=2, space="PSUM") as bigps,
    ):
        ident = sing.tile([SPC, SPC], F32, tag="ident")
        make_identity(nc, ident)

        # ---- input / const DMAs (small first; W last on the ACT ring) ----
        cxg = sing.tile([64, 2048], F32R, tag="cxg")
        nc.sync.dma_start(out=cxg, in_=cxg_d)
        cst = sing.tile([128, 1216], F32R, tag="cst")
        nc.sync.dma_start(out=cst, in_=cst_d)
        v16 = sing.tile([SPC, C], F32, tag="v16")
        nc.sync.dma_start(out=v16, in_=v_d)
        vhl = []
        for g in range(4):
            vg = sing.tile([64, 128], F32R, tag=f"vhl{g}")
            nc.sync.dma_start(out=vg, in_=vhl_d[g, :, :])
            vhl.append(vg)
        tint = sing.tile([33, C], F32R, tag="tint")
        nc.sync.dma_start(out=tint, in_=tint_d)
        bp2 = sing.tile([3, C], F32R, tag="bp2")
        nc.sync.dma_start(out=bp2, in_=bp_d)
        wr = sing.tile([128, NCH, C], F32R, tag="wr")
        nc.scalar.dma_start(out=wr, in_=w_d.rearrange("p (c o) -> p c o", c=NCH))

        # ---- VTO [128, 8, 32]: col 2s = vT_c[:, s], col 2s+1 = 1 ----
        vto = sing.tile([128, NCH, 2 * SPC], F32R, tag="vto")
        nc.scalar.activation(out=vto[:, :, 1:2 * SPC:2], in_=cst[:, 1088:1216],
                             func=COPY)
        for c in range(NCH):
            pt = bigps.tile([128, SPC], F32, tag="big")
            nc.tensor.transpose(pt, v16[:, c * 128:(c + 1) * 128], ident)
            nc.vector.tensor_copy(vto[:, c, 0:2 * SPC:2], pt)

        # ---- node phase: outer product -> exp -> N/D matmuls ----
        e_all = sing.tile([128, SPC, 512], F32R, tag="e_all")
        for s in range(SPC):
            p_s = pps.tile([128, 512], F32, tag="p")
            nc.tensor.matmul(out=p_s, lhsT=vhl[s // 4],
                             rhs=cxg[:, (s % 4) * 512:(s % 4 + 1) * 512],
                             start=True, stop=True)
            nc.scalar.activation(out=e_all[:, s, :], in_=p_s, func=EXP)

        nd_ps = ndps.tile([8, 1024], F32, tag="nd")
        for q in range(4):
            for c in range(NCH):
                nc.tensor.matmul(
                    out=nd_ps[0:8, 256 * q:256 * q + 256],
                    lhsT=vto[:, c, 8 * q:8 * q + 8],
                    rhs=e_all[:, 4 * q:4 * q + 4, c * 64:(c + 1) * 64],
                    start=(c == 0), stop=(c == NCH - 1))
        nd_sb = sing.tile([8, 1024], F32, tag="nd_sb")
        nc.scalar.activation(out=nd_sb, in_=nd_ps, func=COPY)
        # sample s=4q+i: N at partition 2i free 256q+64i, D at partition 2i+1
        # hop 1 (DMA): diagonal extract -> ndx [8, 256]
        ndx = sing.tile([8, 256], F32, tag="ndx")
        for i in range(4):
            nc.sync.dma_start(
                out=ndx[2 * i:2 * i + 2, :].rearrange("p (q k) -> p q k", q=4),
                in_=nd_sb[2 * i:2 * i + 2, :].rearrange(
                    "p (q f) -> p q f", q=4)[:, :, 64 * i:64 * i + 64])
        # DVE pass-through: semaphore between the two dependent DMA hops
        ndx2 = sing.tile([8, 256], F32, tag="ndx2")
        nc.vector.tensor_copy(ndx2, ndx)
        # hop 2: ndx[2i+z, 64q+k] -> ndr[4q+i, 64z+k]  (one DMA per (q, z))
        ndr = sing.tile([SPC, 2 * K], F32, tag="ndr")
        for q in range(4):
            for z in range(2):
                nc.sync.dma_start(
                    out=ndr[4 * q:4 * q + 4, 64 * z:64 * z + 64],
                    in_=ndx2.rearrange("(i z) f -> i z f", i=4)
                    [:, z, 64 * q:64 * q + 64])

        # ---- eval phase part 1: A = t - x, R = 1/A ----
        r_all = sing.tile([128, 8, C], F32R, tag="r_all")
        dinv = sing.tile([SPC, K], F32, tag="dinv")
        for p in range(8):
            a_ps = bigps.tile([128, C], F32, tag="big")
            for h in range(2):
                nc.tensor.matmul(
                    out=a_ps[:, h * 512:(h + 1) * 512],
                    lhsT=cst[0:33, 128 * p:128 * p + 128],
                    rhs=tint[:, h * 512:(h + 1) * 512],
                    start=True, stop=True)
            _recip_fast(nc, r_all[:, p, :], a_ps)
            if p == 1:
                # interleave the small node recip so it is not stuck
                # behind all 8 big recips (DVE FIFO head-of-line)
                _recip_fast(nc, dinv, ndr[:, K:2 * K])

        # ---- node values f -> eval lhsT LW ----
        fv2 = sing.tile([SPC, 2 * K], F32, tag="fv2")
        nc.gpsimd.tensor_mul(fv2[:, 0:K], ndr[:, 0:K], dinv)
        nc.gpsimd.tensor_copy(fv2[:, K:2 * K], fv2[:, 0:K])
        ft_ps = pps.tile([128, SPC], F32, tag="p")
        nc.tensor.transpose(ft_ps, fv2, ident)
        ft2 = sing.tile([128, SPC], F32R, tag="ft2")
        nc.scalar.activation(out=ft2, in_=ft_ps, func=COPY)

        lw = sing.tile([128, 32], F32R, tag="lw")
        nc.gpsimd.tensor_copy(lw, cst[:, 1024:1056].bitcast(F32))
        nc.gpsimd.tensor_scalar_mul(lw[:, 0:32:4], ft2[:, 0:SPC:2].bitcast(F32),
                                    cst[:, 1056:1057].bitcast(F32))
        nc.gpsimd.tensor_scalar_mul(lw[:, 2:32:4], ft2[:, 1:SPC:2].bitcast(F32),
                                    cst[:, 1057:1058].bitcast(F32))

        # ---- eval phase part 2: num/den matmuls + drains ----
        num = sing.tile([SPC, C], F32, tag="num")
        den = sing.tile([SPC, C], F32, tag="den")
        for p in range(8):
            ev = bigps.tile([4, C], F32, tag="big")
            for h in range(2):
                nc.tensor.matmul(
                    out=ev[0:4, h * 512:(h + 1) * 512],
                    lhsT=lw[:, 4 * p:4 * p + 4],
                    rhs=r_all[:, p, h * 512:(h + 1) * 512],
                    start=True, stop=True)
            cr = sing.tile([4, C], F32, tag=f"cr{p % 2}", bufs=1)
            if p % 2 == 0:
                nc.scalar.activation(out=cr, in_=ev, func=COPY)
            else:
                nc.vector.tensor_copy(cr, ev)
            nc.sync.dma_start(out=num[2 * p:2 * p + 2, :], in_=cr[0:4:2, :])
            nc.sync.dma_start(out=den[2 * p:2 * p + 2, :], in_=cr[1:4:2, :])

        dinv2 = sing.tile([SPC, C], F32, tag="dinv2")
        _recip_fast(nc, dinv2, den)
        crossed = sing.tile([SPC, C], F32, tag="crossed")
        nc.vector.tensor_mul(crossed, num, dinv2)
        h_sb = sing.tile([SPC, C], F32, tag="h_sb")
        nc.vector.tensor_add(h_sb, crossed, v16)

        # ---- projection ----
        ht = sing.tile([128, NCH, SPC], F32R, tag="ht")
        for c in range(NCH):
            pt2 = pps.tile([128, SPC], F32, tag="p")
            nc.tensor.transpose(pt2, h_sb[:, c * 128:(c + 1) * 128], ident)
            nc.scalar.activation(out=ht[:, c, :], in_=pt2, func=COPY)

        out_sb = sing.tile([SPC, C], F32, tag="out_sb")
        for hh in range(2):
            op = bigps.tile([SPC, 512], F32, tag="big")
            for c in range(NCH):
                nc.tensor.matmul(
                    out=op, lhsT=ht[:, c, :],
                    rhs=wr[:, c, hh * 512:(hh + 1) * 512],
                    start=(c == 0), stop=False)
            nc.tensor.matmul(
                out=op, lhsT=cst[0:2, 1058:1074],
                rhs=bp2[0:2, hh * 512:(hh + 1) * 512],
                start=False, stop=True)
            yb = sing.tile([SPC, 512], F32, tag=f"yb{hh}")
            nc.vector.tensor_scalar_mul(yb, op, SLOPE)
            nc.vector.tensor_max(out_sb[:, hh * 512:(hh + 1) * 512], op, yb)
        nc.sync.dma_start(out=o_d, in_=out_sb)
        if dbg:
            nc.sync.dma_start(out=dbg["e0"], in_=e_all[:, 0, :].bitcast(F32))
            nc.sync.dma_start(out=dbg["r0"], in_=r_all[:, 0, :].bitcast(F32))
            nc.sync.dma_start(out=dbg["ndr"], in_=ndr)
            nc.sync.dma_start(out=dbg["num"], in_=num)
            nc.sync.dma_start(out=dbg["den"], in_=den)
            nc.sync.dma_start(out=dbg["ndsb"], in_=nd_sb)
            nc.sync.dma_start(out=dbg["ndx"], in_=ndx2)


_NC_CACHE = None
LAST_RESULTS = None


def _prep_host(visual_feat, tactile_feat, conv_w, conv_b, bn_gamma, bn_beta,
               bn_mean, bn_var):
    visual = np.ascontiguousarray(np.asarray(visual_feat, dtype=np.float32))
    tactile = np.ascontiguousarray(np.asarray(tactile_feat, dtype=np.float32))
    conv_w = np.asarray(conv_w, dtype=np.float32)
    conv_b = np.asarray(conv_b, dtype=np.float32)
    gamma = np.asarray(bn_gamma, dtype=np.float32)
    beta = np.asarray(bn_beta, dtype=np.float32)
    mean = np.asarray(bn_mean, dtype=np.float32)
    var = np.asarray(bn_var, dtype=np.float32)

    inv = gamma / np.sqrt(var + np.float32(BN_EPS))        # (C,)
    wc = conv_w[:, :, 1, 1]                                # (O, I) center tap
    wp = (wc * inv[:, None]).T.astype(np.float32)          # (I, O)
    wr = np.ascontiguousarray(
        wp.reshape(NCH, 128, C).transpose(1, 0, 2).reshape(128, NCH * C))
    bp = (((conv_b - mean) * inv) + beta).astype(np.float32)
    bph = _trunc10(bp)
    bp2 = np.ascontiguousarray(
        np.stack([bph, (bp - bph), np.ones(C, np.float32)]).astype(np.float32))

    vhls, tints = [], []
    for k in range(N_CORES):
        sl = slice(k * SPC, (k + 1) * SPC)
        v = visual[sl]
        t = tactile[sl]
        vh, vl = _f32r_split(v.reshape(SPC * NCH, 128))   # rows (s,c)
        vhl = np.zeros((4, 64, 128), np.float32)
        for g in range(4):
            vhl[g, 0:32, :] = vh[32 * g:32 * g + 32]
            vhl[g, 32:64, :] = vl[32 * g:32 * g + 32]
        vhls.append(np.ascontiguousarray(vhl))
        th, tl = _f32r_split(t)
        tint = np.concatenate([th, tl, np.ones((1, C), np.float32)], axis=0)
        tints.append(np.ascontiguousarray(tint))
    return visual, tactile, wr, bp2, vhls, tints


def kernel(visual_feat, tactile_feat, conv_w, conv_b, bn_gamma, bn_beta,
           bn_mean, bn_var):
    global _NC_CACHE, LAST_RESULTS
    visual, tactile, wr, bp2, vhls, tints = _prep_host(
        visual_feat, tactile_feat, conv_w, conv_b, bn_gamma, bn_beta,
        bn_mean, bn_var)

    if _NC_CACHE is None:
        _NC_CACHE = build_nc()

    in_maps = []
    for k in range(N_CORES):
        sl = slice(k * SPC, (k + 1) * SPC)
        in_maps.append({
            "v": np.ascontiguousarray(visual[sl]),
            "vhl": vhls[k],
            "tint": tints[k],
            "w": wr,
            "bp": bp2,
            "cxg": CXG_HOST,
            "cst": CST_HOST,
        })
    res = run_bass_kernel_spmd(
        _NC_CACHE, in_maps, core_ids=list(range(N_CORES)),
        trace=bool(int(os.environ.get("KERNEL_TRACE", "0") or "0")),
    )
    LAST_RESULTS = res
    out = np.concatenate([res.results[k]["o"] for k in range(N_CORES)], axis=0)
    return out.reshape(B, C, 1, 1).astype(np.float32)
